# revision 1
# baseline (speedup 1.0000x reference)
"""Trainium2 Bass kernel for nn_Attention_47553877901998.

GQA attention block: rmsnorm -> q/kv proj -> per-head l2norm*(gamma+1)*sqrt(dh)
-> softcapped causal attention (summing over the 2-query-head group) -> out proj.

Sharding over 8 cores: core c owns batch b = c//4 and kv-heads {2*(c%4), 2*(c%4)+1}
(4 query heads). Each core emits a partial [2048, 1024] output for its batch;
the host sums the 4 partials per batch.

Device-side math notes:
  * norm_w is folded into the projection weights on the host; the rmsnorm row
    scale rs[i] cancels inside the q/k l2norms, so only v is scaled by rs.
  * softcap bounds logits to +-6.25 after the dh**-0.5 scale, so softmax runs
    without max-subtraction; a -1e30 fill before exp handles the causal mask.
  * the walrus build here encodes at most one sem-wait per instruction and
    rejects custom-DVE/TensorTensorReduce ISA structs, so only stock BIR ops
    are used and _split_waits() hoists Tile's extra waits onto NOPs.
"""

import os
import sys

import numpy as np
import ml_dtypes

for _p in ("/root/.axon_site/_ro/trn_rl_repo", "/opt/trn_rl_repo"):
    if os.path.isdir(_p) and _p not in sys.path:
        sys.path.insert(0, _p)

import concourse.bass as bass
import concourse.mybir as mybir
import concourse.tile as tile
from concourse.bass import ds, ts
from concourse.bass_utils import run_bass_kernel_spmd
from concourse.masks import make_identity

F32 = mybir.dt.float32
BF16 = mybir.dt.bfloat16
AF = mybir.ActivationFunctionType
ALU = mybir.AluOpType

B, N, D = 2, 2048, 1024
H, QH, DH = 8, 16, 64
P = 128
NT = N // P              # 16 row tiles
KT = D // P              # 8 contraction tiles
EPS = float(np.finfo(np.float32).eps)


def _split_waits(nc):
    """Hoist all-but-one sync wait per instruction into preceding NOPs.

    The walrus build in this container encodes at most ONE sem-wait per
    instruction ("Too many sync wait commands"); Tile's scheduler attaches
    several. A single-wait NOP on the same engine immediately before the
    instruction preserves the happens-before ordering exactly.
    """
    import bass_rust as _br
    n = 0
    for blk in nc.m.functions[0].blocks:
        out = []
        for ins in blk.instructions:
            si = ins.sync_info
            if si is not None and si.on_wait and len(si.on_wait) > 1:
                waits = list(si.on_wait)
                eng = ins.engine
                for w in waits[:-1]:
                    n += 1
                    out.append(mybir.InstNoOp(
                        name=f"waitsplit-{n}",
                        engine=eng,
                        ins=[], outs=[],
                        sync_info=_br.SyncInfo(on_wait=[w], on_update=[]),
                    ))
                si.on_wait = [waits[-1]]
            out.append(ins)
        blk.instructions = out
    return n


def build_nc(split_waits=True):
    nc = bass.Bass("TRN2")

    tok_d = nc.dram_tensor("tok", [N, D], BF16, kind="ExternalInput")
    wqkv_d = nc.dram_tensor("wqkv", [D, 512], BF16, kind="ExternalInput")
    wout_d = nc.dram_tensor("wout", [2, P, D], BF16, kind="ExternalInput")
    gq_d = nc.dram_tensor("gq", [2, P], F32, kind="ExternalInput")
    gk_d = nc.dram_tensor("gk", [P], F32, kind="ExternalInput")
    out_d = nc.dram_tensor("out_p", [N, D], BF16, kind="ExternalOutput")

    with tile.TileContext(nc) as tc:
        with (
            tc.tile_pool(name="const", bufs=1) as const,
            tc.tile_pool(name="big", bufs=1) as big,
            tc.tile_pool(name="work", bufs=3) as work,
            tc.tile_pool(name="att", bufs=2) as att,
            tc.tile_pool(name="nrm", bufs=2) as nrm,
            tc.tile_pool(name="drp", bufs=2, space="DRAM") as drp,
            tc.tile_pool(name="pps", bufs=2, space="PSUM") as pps,
            tc.tile_pool(name="pot", bufs=1, space="PSUM") as pot,
        ):
            # ---- constants / weights ----
            ident = const.tile([P, P], BF16)
            make_identity(nc, ident)
            epst = const.tile([P, 1], F32)
            nc.vector.memset(epst, EPS)
            wqkv_sb = const.tile([P, KT, 512], BF16)
            nc.sync.dma_start(out=wqkv_sb,
                              in_=wqkv_d.rearrange("(k p) n -> p k n", p=P))
            wout_sb = const.tile([P, 2, D], BF16)
            nc.sync.dma_start(out=wout_sb, in_=wout_d.rearrange("a p n -> p a n"))
            gq_sb = const.tile([P, 2], F32)
            nc.sync.dma_start(out=gq_sb, in_=gq_d.rearrange("a p -> p a"))
            gk_sb = const.tile([P, 1], F32)
            nc.sync.dma_start(out=gk_sb, in_=gk_d[:].unsqueeze(1))

            # ---- phase A: transposed load first (projection feeds off it),
            # then token sumsq ----
            xT = [big.tile([P, N], BF16, tag=f"xT{k}", name=f"xT{k}")
                  for k in range(KT)]
            for k in range(KT):
                nc.sync.dma_start_transpose(out=xT[k], in_=tok_d[:, ts(k, P)])

            ss_all = big.tile([P, NT], F32)
            for t in range(NT):
                tk = work.tile([P, D], BF16, tag="tok")
                nc.sync.dma_start(out=tk, in_=tok_d[ts(t, P), :])
                scr = work.tile([P, D], BF16, tag="scr", bufs=1)
                nc.scalar.activation(scr, tk, AF.Square,
                                     accum_out=ss_all[:, t:t + 1])

            rs_all = big.tile([P, NT], F32)
            srt = big.tile([P, NT], F32)
            nc.scalar.activation(srt, ss_all, AF.Sqrt, bias=epst, scale=1.0 / D)
            nc.vector.reciprocal(rs_all, srt)

            # ---- phase B1: q/kv projection ----
            qk_all = big.tile([P, NT, 384], BF16)     # 4 q heads + 2 k heads
            vext = big.tile([P, NT, 130], BF16)       # [v0 | 1 | v1 | 1]
            ssq_all = big.tile([P, NT, 6], F32)
            rsq_all = big.tile([P, NT, 6], F32)
            srq = big.tile([P, NT * 6], F32)
            for t in range(NT):
                pj = pps.tile([P, 512], F32, tag="ps")
                for k in range(KT):
                    nc.tensor.matmul(pj, lhsT=xT[k][:, ts(t, P)],
                                     rhs=wqkv_sb[:, k, :],
                                     start=(k == 0), stop=(k == KT - 1))
                nc.vector.tensor_copy(qk_all[:, t, :], pj[:, 0:384])
                nc.vector.tensor_scalar_mul(out=vext[:, t, 0:64],
                                            in0=pj[:, 384:448],
                                            scalar1=rs_all[:, t:t + 1])
                nc.vector.tensor_scalar_mul(out=vext[:, t, 65:129],
                                            in0=pj[:, 448:512],
                                            scalar1=rs_all[:, t:t + 1])
                nc.gpsimd.memset(vext[:, t, 64:65], 1.0)
                nc.gpsimd.memset(vext[:, t, 129:130], 1.0)
                sq6 = work.tile([P, 384], F32, tag="sq6", bufs=2)
                nc.scalar.square(sq6, qk_all[:, t, :])
                nc.vector.tensor_reduce(
                    ssq_all[:, t, :], sq6.rearrange("p (h d) -> p h d", d=64),
                    axis=mybir.AxisListType.X, op=ALU.add)
                if t % 4 == 3:
                    g0 = t - 3
                    nc.scalar.activation(
                        srq[:, ds(6 * g0, 24)],
                        ssq_all[:, g0:g0 + 4, :].rearrange("p a b -> p (a b)"),
                        AF.Sqrt, bias=0.0, scale=1.0)
                    nc.vector.reciprocal(
                        rsq_all[:, g0:g0 + 4, :].rearrange("p a b -> p (a b)"),
                        srq[:, ds(6 * g0, 24)])


            # ---- phase B2: l2norm scale + transposes (gamma folded in copy) ----
            # qT[0]: [ (h0,g0) | (h1,g0) ], qT[1]: [ (h0,g1) | (h1,g1) ],
            # kT: [k0 | k1]
            qT = [big.tile([P, N], BF16, tag=f"qT{g}", name=f"qT{g}")
                  for g in range(2)]
            kT = big.tile([P, N], BF16, tag="kT")
            for t in range(NT):
                qn = work.tile([P, 384], BF16, tag="qn")
                for j in range(4):   # q head j -> dest col block
                    dest = 128 * (j % 2) + 64 * (j // 2)
                    nc.scalar.mul(
                        qn[:, ds(dest, 64)],
                        qk_all[:, t, ds(64 * j, 64)],
                        rsq_all[:, t, j:j + 1])
                for j in range(2):   # k heads
                    nc.vector.tensor_scalar_mul(
                        out=qn[:, ds(256 + 64 * j, 64)],
                        in0=qk_all[:, t, ds(256 + 64 * j, 64)],
                        scalar1=rsq_all[:, t, 4 + j:5 + j])
                tp = pps.tile([P, 384], BF16, tag="ps")
                for b3 in range(3):
                    nc.tensor.transpose(tp[:, ds(128 * b3, P)],
                                        qn[:, ds(128 * b3, P)], ident)
                nc.scalar.mul(qT[0][:, ts(t, P)], tp[:, 0:128],
                              gq_sb[:, 0:1])
                nc.scalar.mul(qT[1][:, ts(t, P)], tp[:, 128:256],
                              gq_sb[:, 1:2])
                nc.vector.tensor_scalar_mul(out=kT[:, ts(t, P)],
                                            in0=tp[:, 256:384],
                                            scalar1=gk_sb)

            # ---- phase C: attention, one (head, group) instance at a time ----
            # oT_st holds the UNNORMALIZED attention outputs (g0 rows 0:64,
            # g1 rows 64:128); l_st holds the 2 softmax denominators per head.
            oT_st = [big.tile([P, N], BF16, tag=f"ost{ih}", name=f"ost{ih}")
                     for ih in range(2)]
            oT_nm = [big.tile([P, N], BF16, tag=f"onm{ih}", name=f"onm{ih}")
                     for ih in range(2)]
            HN = N // 2
            for ih in range(2):
                for g in range(2):
                    rows = ds(64 * ih, 64)
                    rdr = drp.tile([1, N], F32, tag="rdr")
                    # two i-halves per instance: [65,1024] psum (2 banks,
                    # double-buffered) so consecutive halves/instances overlap
                    for hf in range(2):
                        lo, hi = HN * hf, HN * (hf + 1)
                        ot = pot.tile([65, HN], F32, tag="ot", bufs=2)
                        njt = 8 * (hf + 1)
                        for jt in range(njt):
                            i_start = max(P * jt, lo)
                            ni = hi - i_start
                            pT = att.tile([P, ni], BF16, tag="pT", bufs=4)
                            cap = att.tile([P, ni], F32, tag="cap", bufs=3)
                            st = pps.tile([P, ni], F32, tag="ps")
                            for hb in range(0, ni, 512):
                                hw = min(512, ni - hb)
                                nc.tensor.matmul(
                                    st[:, ds(hb, hw)],
                                    lhsT=kT[rows, ts(jt, P)],
                                    rhs=qT[g][rows, ds(i_start + hb, hw)],
                                    start=True, stop=True)
                            nc.scalar.activation(cap, st, AF.Tanh, scale=0.02)
                            nc.scalar.activation(pT, cap, AF.Exp, scale=6.25)
                            if i_start == P * jt:
                                # causal mask on the leading diagonal block
                                nc.gpsimd.affine_select(
                                    out=pT[:, 0:P], in_=pT[:, 0:P],
                                    compare_op=ALU.is_ge, fill=0.0,
                                    base=0, pattern=[[1, P]],
                                    channel_multiplier=-1)
                            for c in range(2 * hf, 2 * hf + 2):
                                ic = 512 * c
                                if ic + 512 <= i_start:
                                    continue
                                off = max(0, i_start - ic)
                                nc.tensor.matmul(
                                    ot[:, ds(ic - lo + off, 512 - off)],
                                    lhsT=vext[:, jt, ds(65 * ih, 65)],
                                    rhs=pT[:, ds(ic + off - i_start, 512 - off)],
                                    start=(jt == 0),
                                    stop=(jt == min(njt - 1, 4 * c + 3)))
                        # drain this half: unnormalized output + 1/l to DRAM
                        lrow = nrm.tile([1, HN], F32, tag="lrow", bufs=2)
                        nc.vector.tensor_copy(lrow, ot[64:65, :])
                        rrow = nrm.tile([1, HN], F32, tag="rrow", bufs=2)
                        nc.vector.reciprocal(rrow, lrow)
                        nc.sync.dma_start(
                            out=bass.AP(tensor=rdr.tensor,
                                        offset=rdr.offset + lo,
                                        ap=[[HN, 1], [1, HN]]),
                            in_=rrow)
                        if g == 0:
                            nc.vector.tensor_copy(oT_st[ih][0:64, ds(lo, HN)],
                                                  ot[0:64, :])
                        else:
                            og = nrm.tile([64, HN], BF16, tag="og", bufs=2)
                            nc.vector.tensor_copy(og, ot[0:64, :])
                            nc.sync.dma_start(
                                out=oT_st[ih][64:128, ds(lo, HN)], in_=og)
                    _stash_l(nc, drp, ih, g, rdr)

                # ---- per-head normalization: broadcast 1/l and multiply ----
                rl_bc = nrm.tile([P, N], F32, tag="rlbc", bufs=1)
                for g in range(2):
                    src_ = _lmap[(ih, g)]
                    nc.sync.dma_start(
                        out=rl_bc[ds(64 * g, 64), :].unsqueeze(1),
                        in_=bass.AP(tensor=src_.tensor, offset=src_.offset,
                                    ap=[[0, 64], [N, 1], [1, N]]))
                nc.vector.tensor_mul(oT_nm[ih], oT_st[ih], rl_bc)

            # ---- phase D: output projection (heads accumulate in psum) ----
            for t in range(NT):
                for c in range(2):
                    op_ps = pps.tile([P, 512], F32, tag="ps")
                    for ih in range(2):
                        nc.tensor.matmul(op_ps,
                                         lhsT=oT_nm[ih][:, ts(t, P)],
                                         rhs=wout_sb[:, ih, ds(512 * c, 512)],
                                         start=(ih == 0), stop=(ih == 1))
                    ob = work.tile([P, 512], BF16, tag="ob")
                    if c == 0:
                        nc.scalar.copy(ob, op_ps)
                    else:
                        nc.vector.tensor_copy(ob, op_ps)
                    nc.sync.dma_start(out=out_d[ts(t, P), ds(512 * c, 512)],
                                      in_=ob)

    if split_waits:
        _split_waits(nc)
    return nc


_lmap = {}


def _stash_l(nc, drp, ih, g, ldr):
    _lmap[(ih, g)] = ldr


_NC_CACHE = {}


def _get_nc():
    if "nc" not in _NC_CACHE:
        _lmap.clear()
        _NC_CACHE["nc"] = build_nc()
    return _NC_CACHE["nc"]


def _make_in_maps(inputs):
    tokens = np.asarray(inputs["tokens"], np.float32)
    norm_w = np.asarray(inputs["norm_w"], np.float32)
    Wq = np.asarray(inputs["Wq"], np.float32)
    Wkv = np.asarray(inputs["Wkv"], np.float32)
    Wout = np.asarray(inputs["Wout"], np.float32)
    qg = np.asarray(inputs["q_gamma"], np.float32)
    kg = np.asarray(inputs["k_gamma"], np.float32)

    bf = ml_dtypes.bfloat16
    sq = np.sqrt(np.float32(DH))
    tok_bf = [tokens[b].astype(bf) for b in range(B)]
    wq_n = norm_w[:, None] * Wq
    wkv_n = norm_w[:, None] * Wkv

    in_maps = []
    for c in range(8):
        b, hp = c // 4, c % 4
        h0, h1 = 2 * hp, 2 * hp + 1
        qh = 4 * hp
        wqkv = np.concatenate([
            wq_n[:, 64 * qh:64 * (qh + 4)],
            wkv_n[:, 64 * h0:64 * (h1 + 1)],
            wkv_n[:, 512 + 64 * h0:512 + 64 * (h1 + 1)],
        ], axis=1).astype(bf)                                   # [1024, 512]
        wout = np.stack([
            np.concatenate([Wout[64 * h:64 * (h + 1)]] * 2, 0)  # [128, 1024]
            for h in (h0, h1)]).astype(bf)
        gqs = (qg + 1.0) * sq
        gks = (kg + 1.0) * sq
        # qT tile A rows: [(h0,g0) | (h1,g0)]; tile B: g=1
        gq_in = np.stack([
            np.concatenate([gqs[qh + 0], gqs[qh + 2]]),
            np.concatenate([gqs[qh + 1], gqs[qh + 3]]),
        ])
        gk_in = np.concatenate([gks[h0], gks[h1]])              # [128]
        in_maps.append({
            "tok": np.ascontiguousarray(tok_bf[b]),
            "wqkv": np.ascontiguousarray(wqkv),
            "wout": np.ascontiguousarray(wout),
            "gq": np.ascontiguousarray(gq_in.astype(np.float32)),
            "gk": np.ascontiguousarray(gk_in.astype(np.float32)),
        })
    return in_maps


def _run(inputs, **kw):
    nc = _get_nc()
    in_maps = _make_in_maps(inputs)
    res = run_bass_kernel_spmd(nc, in_maps, core_ids=list(range(8)), **kw)
    out = np.zeros((B, N, D), np.float32)
    for c in range(8):
        out[c // 4] += res.results[c]["out_p"].astype(np.float32)
    return out, res


def kernel(**inputs) -> np.ndarray:
    out, _ = _run(inputs)
    return out


if __name__ == "__main__":
    import reference as R
    inp = {k: np.asarray(v) for k, v in R.setup_inputs().items()}
    exp = np.asarray(R.reference(**inp))
    got = kernel(**inp)
    rel = np.linalg.norm(got - exp) / np.linalg.norm(exp)
    print("Relative error:", rel)



# revision 58
# speedup vs baseline: 1.2186x; 1.2186x over previous
"""Trainium2 Bass kernel for nn_Attention_47553877901998.

GQA attention block: rmsnorm -> q/kv proj -> per-head l2norm*(gamma+1)*sqrt(dh)
-> softcapped causal attention (summing over the 2-query-head group) -> out proj.

Sharding over 8 cores: core c owns batch b = c//4 and kv-heads {2*(c%4), 2*(c%4)+1}
(4 query heads). Each core emits a partial [2048, 1024] output for its batch;
the host sums the 4 partials per batch.

Device-side math notes:
  * norm_w is folded into the projection weights on the host; the rmsnorm row
    scale rs[i] cancels inside the q/k l2norms, so only v is scaled by rs.
  * softcap bounds logits to +-6.25 after the dh**-0.5 scale, so softmax runs
    without max-subtraction; a -1e30 fill before exp handles the causal mask.
  * the walrus build here encodes at most one sem-wait per instruction and
    rejects custom-DVE/TensorTensorReduce ISA structs, so only stock BIR ops
    are used and _split_waits() hoists Tile's extra waits onto NOPs.

Engine budget (cost-model): the Activation engine owns only the attention
tanh/exp (~150us) plus four tiny Sqrt batches; token sumsq runs as DVE
squares + a ones-row PE matmul, head norms as DVE square+reduce, all
psum->sbuf drains on Pool, and the kernel is pipelined in two token halves
so projection/outproj overlap the attention phase.
"""

import os
import sys

import numpy as np
import ml_dtypes

for _p in ("/root/.axon_site/_ro/trn_rl_repo", "/opt/trn_rl_repo"):
    if os.path.isdir(_p) and _p not in sys.path:
        sys.path.insert(0, _p)

import concourse.bass as bass
import concourse.mybir as mybir
import concourse.tile as tile
from concourse.bass import ds, ts
from concourse.bass_utils import run_bass_kernel_spmd
from concourse.masks import make_identity

F32 = mybir.dt.float32
BF16 = mybir.dt.bfloat16
AF = mybir.ActivationFunctionType
ALU = mybir.AluOpType

B, N, D = 2, 2048, 1024
H, QH, DH = 8, 16, 64
P = 128
NT = N // P              # 16 row tiles
KT = D // P              # 8 contraction tiles
HN = N // 2
EPS = float(np.finfo(np.float32).eps)


def _split_waits(nc):
    """Hoist all-but-one sync wait per instruction into preceding NOPs.

    The walrus build in this container encodes at most ONE sem-wait per
    instruction ("Too many sync wait commands"); Tile's scheduler attaches
    several. A single-wait NOP on the same engine immediately before the
    instruction preserves the happens-before ordering exactly.
    """
    import bass_rust as _br
    n = 0
    for blk in nc.m.functions[0].blocks:
        out = []
        for ins in blk.instructions:
            si = ins.sync_info
            if si is not None and si.on_wait and len(si.on_wait) > 1:
                waits = list(si.on_wait)
                eng = ins.engine
                for w in waits[:-1]:
                    n += 1
                    out.append(mybir.InstNoOp(
                        name=f"waitsplit-{n}",
                        engine=eng,
                        ins=[], outs=[],
                        sync_info=_br.SyncInfo(on_wait=[w], on_update=[]),
                    ))
                si.on_wait = [waits[-1]]
            out.append(ins)
        blk.instructions = out
    return n


def build_nc(split_waits=True):
    nc = bass.Bass("TRN2")

    tok_d = nc.dram_tensor("tok", [N, D], BF16, kind="ExternalInput")
    wqkv_d = nc.dram_tensor("wqkv", [D, 512], BF16, kind="ExternalInput")
    wout_d = nc.dram_tensor("wout", [2, P, D], BF16, kind="ExternalInput")
    gq_d = nc.dram_tensor("gq", [2, P], F32, kind="ExternalInput")
    gk_d = nc.dram_tensor("gk", [P], F32, kind="ExternalInput")
    out_d = nc.dram_tensor("out_p", [N, D], BF16, kind="ExternalOutput")

    with tile.TileContext(nc) as tc:
        with (
            tc.tile_pool(name="const", bufs=1) as const,
            tc.tile_pool(name="big", bufs=1) as big,
            tc.tile_pool(name="work", bufs=3) as work,
            tc.tile_pool(name="att", bufs=2) as att,
            tc.tile_pool(name="nrm", bufs=2) as nrm,
            tc.tile_pool(name="drp", bufs=2, space="DRAM") as drp,
            tc.tile_pool(name="pps", bufs=2, space="PSUM") as pps,
            tc.tile_pool(name="ppj", bufs=2, space="PSUM") as ppj,
            tc.tile_pool(name="pot", bufs=1, space="PSUM") as pot,
        ):
            # ---- constants (weight DMAs are emitted after the first token
            # loads: the DMA engines are a serial resource and the token
            # transpose loads head the critical path) ----
            ident = const.tile([P, P], BF16)
            make_identity(nc, ident)
            epst = const.tile([P, 1], F32)
            nc.vector.memset(epst, EPS)
            ones1 = const.tile([P, 1], BF16)
            nc.vector.memset(ones1, 1.0)
            wqkv_sb = const.tile([P, KT, 512], BF16)
            wout_sb = const.tile([P, 2, D], BF16)
            gq_sb = const.tile([P, 2], F32)
            gk_sb = const.tile([P, 1], F32)

            def load_weights():
                nc.sync.dma_start(
                    out=wqkv_sb,
                    in_=wqkv_d.rearrange("(k p) n -> p k n", p=P))
                nc.sync.dma_start(
                    out=wout_sb, in_=wout_d.rearrange("a p n -> p a n"))
                nc.sync.dma_start(out=gq_sb,
                                  in_=gq_d.rearrange("a p -> p a"))
                nc.sync.dma_start(out=gk_sb, in_=gk_d[:].unsqueeze(1))

            # ---- persistent tiles ----
            xT = [big.tile([P, N], BF16, tag=f"xT{k}", name=f"xT{k}")
                  for k in range(KT)]
            xsq = [big.tile([P, N], BF16, tag=f"xsq{k}", name=f"xsq{k}")
                   for k in range(KT)]
            rs_row = big.tile([1, N], F32)
            srt_all = big.tile([P, NT], F32)
            rs_all = big.tile([P, NT], F32)
            qk_all = big.tile([P, NT, 384], BF16)     # 4 q heads + 2 k heads
            vext = big.tile([P, NT, 130], BF16)       # [v0 | 1 | v1 | 1]
            ssq_all = big.tile([P, NT, 6], F32)
            rsq_all = big.tile([P, NT, 6], F32)
            srq = big.tile([P, NT * 6], F32)
            # qT[0]: [ (h0,g0) | (h1,g0) ], qT[1]: g=1; kT: [k0 | k1]
            qT = [big.tile([P, N], BF16, tag=f"qT{g}", name=f"qT{g}")
                  for g in range(2)]
            kT = big.tile([P, N], BF16, tag="kT")
            # unnormalized attention outputs; g0 rows 0:64, g1 rows 64:128
            # (normalized in place per half once 1/l is known)
            oT_st = [big.tile([P, N], BF16, tag=f"ost{ih}", name=f"ost{ih}")
                     for ih in range(2)]
            # reciprocal softmax denominators, one row per (head, group)
            # (single-partition engine ops must start at partition 0)
            lq = [big.tile([1, N], F32, tag=f"lq{r}", name=f"lq{r}")
                  for r in range(4)]

            # ones columns of vext (65th row trick for the denominators)
            nc.gpsimd.memset(vext[:, :, 64:65], 1.0)
            nc.gpsimd.memset(vext[:, :, 129:130], 1.0)

            # warm-up: keep the PE p-state ramp hot while the token DMAs
            # land (matmuls into a scratch accumulator; result discarded)
            wrm = const.tile([P, 512], BF16)
            nc.vector.memset(wrm, 0.0)
            wps = ppj.tile([P, 512], F32, tag="pj", name="warm")
            NWARM = 80
            for i in range(NWARM):
                nc.tensor.matmul(wps, lhsT=ident[:, 0:P], rhs=wrm,
                                 start=(i == 0), stop=(i == NWARM - 1))
            wdr = work.tile([1, 512], F32, tag="wdr", bufs=1)
            nc.vector.tensor_copy(wdr, wps[0:1, :])

            dmae = [nc.sync, nc.vector, nc.gpsimd, nc.scalar]

            def emit_prologue(h):
                """Loads, token sumsq, projection and q/k norm for one half.

                A generator: yields between small chunks so the caller can
                interleave this work into the other half's attention stream
                (PE executes instructions in issue order per engine).
                """
                lo = HN * h
                for k in range(KT):
                    nc.sync.dma_start_transpose(
                        out=xT[k][:, ds(lo, HN)],
                        in_=tok_d[ds(lo, HN), ts(k, P)])
                    yield
                if h == 0:
                    load_weights()
                for k in range(KT):
                    nc.vector.tensor_tensor(
                        out=xsq[k][:, ds(lo, HN)], in0=xT[k][:, ds(lo, HN)],
                        in1=xT[k][:, ds(lo, HN)], op=ALU.mult)
                    yield

                def token_sumsq():
                    # ones-row matmul sumsq + sqrt + bounce to [128, NT]
                    for c in (2 * h, 2 * h + 1):
                        ssp = ppj.tile([1, 512], F32, tag="pj",
                                       name=f"ssp{c}")
                        for k in range(KT):
                            nc.tensor.matmul(ssp, lhsT=ones1,
                                             rhs=xsq[k][:, ds(512 * c, 512)],
                                             start=(k == 0),
                                             stop=(k == KT - 1))
                            yield
                        nc.scalar.activation(rs_row[:, ds(512 * c, 512)],
                                             ssp, AF.Sqrt, bias=epst[0:1, :],
                                             scale=1.0 / D)
                        yield
                    rs_dr = drp.tile([1, HN], F32, tag="rsdr")
                    nc.sync.dma_start(out=rs_dr, in_=rs_row[:, ds(lo, HN)])
                    nc.sync.dma_start(
                        out=srt_all[:, ds(8 * h, 8)],
                        in_=bass.AP(tensor=rs_dr.tensor, offset=rs_dr.offset,
                                    ap=[[1, P], [P, 8]]))
                    nc.vector.reciprocal(rs_all[:, ds(8 * h, 8)],
                                         srt_all[:, ds(8 * h, 8)])
                    yield

                sumsq_gen = token_sumsq()
                # 4-tile batches: all B1 chains first, then the norm scale,
                # then B2 — keeps the in-order PE stream free of transposes
                # that wait on sqrt results of later projections.
                for tb in (8 * h, 8 * h + 4):
                    for t in range(tb, tb + 4):
                        # B1: projection + v scale + head sumsq
                        pj = ppj.tile([P, 512], F32, tag="pj", name=f"pj{t}")
                        for k in range(KT):
                            nc.tensor.matmul(pj, lhsT=xT[k][:, ts(t, P)],
                                             rhs=wqkv_sb[:, k, :],
                                             start=(k == 0),
                                             stop=(k == KT - 1))
                            yield
                        nc.vector.tensor_copy(qk_all[:, t, :], pj[:, 0:384])
                        # copy v unscaled so pj's release does not wait on
                        # the rs bounce; scale in place below
                        nc.vector.tensor_copy(vext[:, t, 0:64],
                                              pj[:, 384:448])
                        nc.vector.tensor_copy(vext[:, t, 65:129],
                                              pj[:, 448:512])
                        yield
                        sq6 = work.tile([P, 384], BF16, tag="sq6", bufs=2)
                        nc.vector.tensor_tensor(
                            out=sq6, in0=qk_all[:, t, :],
                            in1=qk_all[:, t, :], op=ALU.mult)
                        nc.vector.tensor_reduce(
                            ssq_all[:, t, :],
                            sq6.rearrange("p (h d) -> p h d", d=64),
                            axis=mybir.AxisListType.X, op=ALU.add)
                        yield
                    if tb == 8 * h:
                        # token sumsq chains slot in after the first B1
                        # batch so they don't gate the pj pool rotation
                        yield from sumsq_gen
                    nc.scalar.activation(
                        srq[:, ds(6 * tb, 24)],
                        ssq_all[:, tb:tb + 4, :].rearrange("p a b -> p (a b)"),
                        AF.Sqrt, bias=0.0, scale=1.0)
                    nc.vector.reciprocal(
                        rsq_all[:, tb:tb + 4, :].rearrange("p a b -> p (a b)"),
                        srq[:, ds(6 * tb, 24)])
                    yield
                    for t in range(tb, tb + 4):
                        # rmsnorm scale on v (in place, off the pj critical
                        # path)
                        nc.vector.tensor_scalar_mul(
                            out=vext[:, t, 0:64], in0=vext[:, t, 0:64],
                            scalar1=rs_all[:, t:t + 1])
                        nc.vector.tensor_scalar_mul(
                            out=vext[:, t, 65:129], in0=vext[:, t, 65:129],
                            scalar1=rs_all[:, t:t + 1])
                    yield
                    for t in range(tb, tb + 4):
                        # B2: l2norm scale + transpose (gamma in the copy)
                        qn = work.tile([P, 384], BF16, tag="qn")
                        for j in range(4):   # q head j -> dest col block
                            dest = 128 * (j % 2) + 64 * (j // 2)
                            nc.vector.tensor_scalar_mul(
                                out=qn[:, ds(dest, 64)],
                                in0=qk_all[:, t, ds(64 * j, 64)],
                                scalar1=rsq_all[:, t, j:j + 1])
                        for j in range(2):   # k heads
                            nc.vector.tensor_scalar_mul(
                                out=qn[:, ds(256 + 64 * j, 64)],
                                in0=qk_all[:, t, ds(256 + 64 * j, 64)],
                                scalar1=rsq_all[:, t, 4 + j:5 + j])
                        yield
                        tp = ppj.tile([P, 384], BF16, tag="pj", name=f"tp{t}")
                        for b3 in range(3):
                            nc.tensor.transpose(tp[:, ds(128 * b3, P)],
                                                qn[:, ds(128 * b3, P)], ident)
                            yield
                        nc.vector.tensor_scalar_mul(out=qT[0][:, ts(t, P)],
                                                    in0=tp[:, 0:128],
                                                    scalar1=gq_sb[:, 0:1])
                        nc.vector.tensor_scalar_mul(out=qT[1][:, ts(t, P)],
                                                    in0=tp[:, 128:256],
                                                    scalar1=gq_sb[:, 1:2])
                        nc.vector.tensor_scalar_mul(out=kT[:, ts(t, P)],
                                                    in0=tp[:, 256:384],
                                                    scalar1=gk_sb)
                        yield

            def emit_quarter_norm(c):
                """Bounce 1/l rows through DRAM, broadcast, normalize oT."""
                qcols = ds(512 * c, 512)
                ldr = drp.tile([4, 512], F32, tag="ldr", name=f"ldr{c}")
                for r in range(4):
                    nc.sync.dma_start(out=ldr[r:r + 1, :],
                                      in_=lq[r][:, qcols])
                for ih in range(2):
                    rl_bc = nrm.tile([P, 512], F32, tag="rlbc", bufs=2,
                                     name=f"rlbc{c}_{ih}")
                    nc.sync.dma_start(
                        out=rl_bc,
                        in_=bass.AP(tensor=ldr.tensor,
                                    offset=ldr.offset + 2 * ih * 512,
                                    ap=[[512, 2], [0, 64], [1, 512]]))
                    nc.vector.tensor_tensor(
                        out=oT_st[ih][:, qcols], in0=oT_st[ih][:, qcols],
                        in1=rl_bc, op=ALU.mult)

            def emit_quarter_proj(c, deep=False):
                """Output projection for one normalized 512-query block.

                deep=True (the post-attention quarters) widens the psum
                rotation to 4 tiles and spreads drains over three engines so
                the tail pipeline isn't gated on two psum buffers.
                """
                for t in range(4 * c, 4 * c + 4):
                    for c2 in range(2):
                        i = 2 * (t - 4 * c) + c2
                        if not deep:
                            pool, tag = ppj, "pj"
                        else:
                            pool, tag = [(ppj, "pj"), (pps, "ps"),
                                         (pot, "ot")][i % 3]
                        op_ps = pool.tile([P, 512], F32, tag=tag,
                                          bufs=2, name=f"od{t}_{c2}")
                        for ih in range(2):
                            nc.tensor.matmul(
                                op_ps,
                                lhsT=oT_st[ih][:, ts(t, P)],
                                rhs=wout_sb[:, ih, ds(512 * c2, 512)],
                                start=(ih == 0), stop=(ih == 1))
                        ob = work.tile([P, 512], BF16, tag="ob")
                        # GPSIMD cannot access PSUM on this hw build, so
                        # drains go to DVE (and the idle ACT engine in the
                        # tail; Copy shares the exp activation table)
                        if not deep:
                            nc.vector.tensor_copy(ob, op_ps)
                        elif i % 2 == 0:
                            nc.scalar.copy(ob, op_ps)
                        else:
                            nc.vector.tensor_copy(ob, op_ps)
                        [nc.sync, nc.scalar][i % 2].dma_start(
                            out=out_d[ts(t, P), ds(512 * c2, 512)], in_=ob)

            def attention_steps(h, closed):
                """Software-pipelined attention for query half h.

                Emits scores+tanh+exp for step s+1 before the PV matmuls of
                step s, so the in-order PE stream never stalls on the
                Activation engine at instance boundaries. Appends each
                512-query quarter index to `closed` once its last PV + l row
                is emitted.
                """
                lo, hi = HN * h, HN * (h + 1)
                njt = 8 * (h + 1)
                steps = [(ih, g, jt)
                         for ih in range(2) for g in range(2)
                         for jt in range(njt)]
                otc = {}      # (ih, g) -> {c: psum tile}
                pend = {}     # step -> pT tile

                def scores(step):
                    ih, g, jt = step
                    rows = ds(64 * ih, 64)
                    i_start = max(P * jt, lo)
                    ni = hi - i_start
                    pT = att.tile([P, ni], BF16, tag="pT", bufs=4,
                                  name=f"pT{h}_{ih}{g}{jt}")
                    st = pps.tile([P, ni], F32, tag="ps",
                                  name=f"st{h}_{ih}{g}{jt}")
                    for hb in range(0, ni, 512):
                        hw = min(512, ni - hb)
                        nc.tensor.matmul(
                            st[:, ds(hb, hw)],
                            lhsT=kT[rows, ts(jt, P)],
                            rhs=qT[g][rows, ds(i_start + hb, hw)],
                            start=True, stop=True)
                    nc.scalar.activation(st, st, AF.Tanh, scale=0.02)
                    nc.scalar.activation(pT, st, AF.Exp, scale=6.25)
                    if i_start == P * jt:
                        # causal mask on the leading diagonal block
                        nc.gpsimd.affine_select(
                            out=pT[:, 0:P], in_=pT[:, 0:P],
                            compare_op=ALU.is_ge, fill=0.0,
                            base=0, pattern=[[1, P]],
                            channel_multiplier=-1)
                    pend[step] = pT

                def pv(step):
                    ih, g, jt = step
                    pT = pend.pop(step)
                    i_start = max(P * jt, lo)
                    oc = otc.setdefault((ih, g), {})
                    for c in (2 * h, 2 * h + 1):
                        ic = 512 * c
                        if ic + 512 <= i_start:
                            continue
                        off = max(0, i_start - ic)
                        if c not in oc:
                            oc[c] = pot.tile([65, 512], F32, tag="ot",
                                             bufs=2, name=f"ot{h}_{ih}{g}{c}")
                        last = min(njt - 1, 4 * c + 3)
                        nc.tensor.matmul(
                            oc[c][:, ds(off, 512 - off)],
                            lhsT=vext[:, jt, ds(65 * ih, 65)],
                            rhs=pT[:, ds(ic + off - i_start, 512 - off)],
                            start=(jt == 0), stop=(jt == last))
                        if jt == last:
                            # drain this 512-query quarter
                            qcols = ds(ic, 512)
                            if g == 0:
                                nc.vector.tensor_copy(
                                    oT_st[ih][0:64, qcols], oc[c][0:64, :])
                            else:
                                og = nrm.tile([64, 512], BF16,
                                              tag="og", bufs=2,
                                              name=f"og{h}_{ih}{c}")
                                nc.vector.tensor_copy(og, oc[c][0:64, :])
                                nc.gpsimd.dma_start(
                                    out=oT_st[ih][64:128, qcols], in_=og)
                            nc.vector.reciprocal(
                                lq[2 * ih + g][:, qcols], oc[c][64:65, :])
                            if (ih, g) == (1, 1):
                                closed.append(c)

                for i, s in enumerate(steps):
                    scores(s)
                    if i >= 1:
                        pv(steps[i - 1])
                    yield
                pv(steps[-1])

            def drain(gen):
                for _ in gen:
                    pass

            # ---- half 0 prologue runs up front ----
            drain(emit_prologue(0))

            # ---- half-0 attention with half-1 prologue interleaved ----
            pro1 = emit_prologue(1)
            closed = []
            for _ in attention_steps(0, closed):
                for _ in range(5):
                    next(pro1, None)
            drain(pro1)

            # ---- half-1 attention; finished-quarter outproj interleaved
            # (each quarter's 1/l + normalize + outproj fires two steps
            # after its last PV, overlapping the remaining attention) ----
            nstep = 0
            emit_at = 2
            done = []
            for _ in attention_steps(1, closed):
                nstep += 1
                if closed and nstep >= emit_at:
                    c = closed.pop(0)
                    emit_quarter_norm(c)
                    emit_quarter_proj(c)
                    done.append(c)
                    emit_at = nstep + 6
            # tail quarters: launch all 1/l bounces first, then project
            for c in closed:
                emit_quarter_norm(c)
            for c in closed:
                emit_quarter_proj(c, deep=True)

    if split_waits:
        _split_waits(nc)
    return nc


_NC_CACHE = {}


def _get_nc():
    if "nc" not in _NC_CACHE:
        _NC_CACHE["nc"] = build_nc()
    return _NC_CACHE["nc"]


def _make_in_maps(inputs):
    tokens = np.asarray(inputs["tokens"], np.float32)
    norm_w = np.asarray(inputs["norm_w"], np.float32)
    Wq = np.asarray(inputs["Wq"], np.float32)
    Wkv = np.asarray(inputs["Wkv"], np.float32)
    Wout = np.asarray(inputs["Wout"], np.float32)
    qg = np.asarray(inputs["q_gamma"], np.float32)
    kg = np.asarray(inputs["k_gamma"], np.float32)

    bf = ml_dtypes.bfloat16
    sq = np.sqrt(np.float32(DH))
    tok_bf = [tokens[b].astype(bf) for b in range(B)]
    wq_n = norm_w[:, None] * Wq
    wkv_n = norm_w[:, None] * Wkv

    in_maps = []
    for c in range(8):
        b, hp = c // 4, c % 4
        h0, h1 = 2 * hp, 2 * hp + 1
        qh = 4 * hp
        wqkv = np.concatenate([
            wq_n[:, 64 * qh:64 * (qh + 4)],
            wkv_n[:, 64 * h0:64 * (h1 + 1)],
            wkv_n[:, 512 + 64 * h0:512 + 64 * (h1 + 1)],
        ], axis=1).astype(bf)                                   # [1024, 512]
        wout = np.stack([
            np.concatenate([Wout[64 * h:64 * (h + 1)]] * 2, 0)  # [128, 1024]
            for h in (h0, h1)]).astype(bf)
        gqs = (qg + 1.0) * sq
        gks = (kg + 1.0) * sq
        # qT tile A rows: [(h0,g0) | (h1,g0)]; tile B: g=1
        gq_in = np.stack([
            np.concatenate([gqs[qh + 0], gqs[qh + 2]]),
            np.concatenate([gqs[qh + 1], gqs[qh + 3]]),
        ])
        gk_in = np.concatenate([gks[h0], gks[h1]])              # [128]
        in_maps.append({
            "tok": np.ascontiguousarray(tok_bf[b]),
            "wqkv": np.ascontiguousarray(wqkv),
            "wout": np.ascontiguousarray(wout),
            "gq": np.ascontiguousarray(gq_in.astype(np.float32)),
            "gk": np.ascontiguousarray(gk_in.astype(np.float32)),
        })
    return in_maps


def _run(inputs, **kw):
    nc = _get_nc()
    in_maps = _make_in_maps(inputs)
    res = run_bass_kernel_spmd(nc, in_maps, core_ids=list(range(8)), **kw)
    out = np.zeros((B, N, D), np.float32)
    for c in range(8):
        out[c // 4] += res.results[c]["out_p"].astype(np.float32)
    return out, res


def kernel(**inputs) -> np.ndarray:
    out, _ = _run(inputs)
    return out


if __name__ == "__main__":
    import reference as R
    inp = {k: np.asarray(v) for k, v in R.setup_inputs().items()}
    exp = np.asarray(R.reference(**inp))
    got = kernel(**inp)
    rel = np.linalg.norm(got - exp) / np.linalg.norm(exp)
    print("Relative error:", rel)
